# revision 1
# baseline (speedup 1.0000x reference)
"""Trainium2 Bass kernel for nn_KerasSeq2Seq: 2-layer LSTM encoder (T=64) +
2-layer LSTM decoder (SEG=32) + Dense(1), B=1024, H=512, F=121.

Sharding: data-parallel over batch across 8 NeuronCores (128 rows each),
weights replicated. Per core, per step, gate pre-activations are computed as
PSUM-accumulated matmuls with the *transposed* hidden state as the stationary
operand; hidden states are re-transposed each step on the tensor engine.
"""

import sys
from contextlib import ExitStack

import numpy as np

sys.path.insert(0, "/opt/trn_rl_repo")

import concourse.bass as bass  # noqa: E402
import concourse.tile as tile  # noqa: E402
from concourse import bacc, mybir  # noqa: E402

N_CORES = 8
B, T_ENC, F, H, SEG = 1024, 64, 121, 512, 32
BL = B // N_CORES            # 128 batch rows per core
GH = 4 * H                   # 2048 gate columns
NKH = H // 128               # 4 K-chunks for an H-dim contraction
FP32 = mybir.dt.float32
AF = mybir.ActivationFunctionType
ALU = mybir.AluOpType

_RUNTIME = {}


def _build_program(t_enc, seg):
    nc = bacc.Bacc("TRN2", target_bir_lowering=False, debug=False,
                   num_devices=N_CORES)

    xT = nc.dram_tensor("xT", [128, t_enc * 128], FP32, kind="ExternalInput").ap()
    w_e0 = nc.dram_tensor("w_e0", [128, GH], FP32, kind="ExternalInput").ap()
    u_e0 = nc.dram_tensor("u_e0", [128, NKH * GH], FP32, kind="ExternalInput").ap()
    w_e1 = nc.dram_tensor("w_e1", [128, NKH * GH], FP32, kind="ExternalInput").ap()
    u_e1 = nc.dram_tensor("u_e1", [128, NKH * GH], FP32, kind="ExternalInput").ap()
    u_d0 = nc.dram_tensor("u_d0", [128, NKH * GH], FP32, kind="ExternalInput").ap()
    w_d1 = nc.dram_tensor("w_d1", [128, NKH * GH], FP32, kind="ExternalInput").ap()
    u_d1 = nc.dram_tensor("u_d1", [128, NKH * GH], FP32, kind="ExternalInput").ap()
    ident = nc.dram_tensor("ident", [128, 128], FP32, kind="ExternalInput").ap()
    dwb = nc.dram_tensor("dwb", [128, H], FP32, kind="ExternalInput").ap()
    out = nc.dram_tensor("out", [128, seg], FP32, kind="ExternalOutput").ap()

    with tile.TileContext(nc) as tc, ExitStack() as ctx:
        wpool = ctx.enter_context(tc.tile_pool(name="w", bufs=1))
        zpool = ctx.enter_context(
            tc.tile_pool(name="z", bufs=6, space=bass.MemorySpace.PSUM))
        trpool = ctx.enter_context(
            tc.tile_pool(name="tr", bufs=2, space=bass.MemorySpace.PSUM))
        gpool = ctx.enter_context(tc.tile_pool(name="g", bufs=8))
        tpool = ctx.enter_context(tc.tile_pool(name="tmp", bufs=3))
        spool = ctx.enter_context(tc.tile_pool(name="state", bufs=1))

        def load(dram_ap, cols, tag, nsplit):
            t = wpool.tile([128, cols], FP32, tag=tag)
            w = cols // nsplit
            for i in range(nsplit):
                nc.sync.dma_start(t[:, i * w:(i + 1) * w],
                                  dram_ap[:, i * w:(i + 1) * w])
            return t

        xT_sb = load(xT, t_enc * 128, "xT", min(4, t_enc))
        we0_sb = load(w_e0, GH, "we0", 2)
        ue0_sb = load(u_e0, NKH * GH, "u0", 8)
        we1_sb = load(w_e1, NKH * GH, "w1", 8)
        ue1_sb = load(u_e1, NKH * GH, "u1", 8)
        id_sb = wpool.tile([128, 128], FP32, tag="ident")
        nc.sync.dma_start(id_sb[:], ident[:])
        dwb_sb = wpool.tile([128, H], FP32, tag="dwb")
        nc.sync.dma_start(dwb_sb[:], dwb[:])

        h0T = spool.tile([128, H], FP32, tag="h0T")
        h1T = spool.tile([128, H], FP32, tag="h1T")
        c0 = spool.tile([128, H], FP32, tag="c0")
        c1 = spool.tile([128, H], FP32, tag="c1")
        out_sb = spool.tile([128, seg], FP32, tag="out")
        for s in (h0T, h1T, c0, c1):
            nc.vector.memset(s[:], 0.0)

        def lstm_gates(ins, c, dense_to=None):
            """Matmuls + activations + c/h update. Returns the h tile.
            ins: list of (lhs_fn(k) -> AP[128,128], rhs_fn(k, n) -> AP[128,512], kc)
            """
            tot = sum(kc for _, _, kc in ins)
            gates = []
            for n in range(4):
                z = zpool.tile([128, 512], FP32, tag="z")
                cnt = 0
                for (lhs_fn, rhs_fn, kc) in ins:
                    for k in range(kc):
                        cnt += 1
                        nc.tensor.matmul(z[:], lhs_fn(k), rhs_fn(k, n),
                                         start=(cnt == 1), stop=(cnt == tot))
                g_t = gpool.tile([128, 512], FP32, tag="gate")
                nc.scalar.activation(g_t[:], z[:],
                                     AF.Tanh if n == 2 else AF.Sigmoid)
                gates.append(g_t)
            i_t, f_t, g_t, o_t = gates
            ig = tpool.tile([128, 512], FP32, tag="ig")
            nc.vector.tensor_mul(ig[:], i_t[:], g_t[:])
            nc.vector.tensor_mul(c[:], f_t[:], c[:])
            nc.vector.tensor_add(c[:], c[:], ig[:])
            tc_t = tpool.tile([128, 512], FP32, tag="tc")
            nc.scalar.activation(tc_t[:], c[:], AF.Tanh)
            h = tpool.tile([128, 512], FP32, tag="h")
            nc.vector.tensor_mul(h[:], o_t[:], tc_t[:])
            if dense_to is not None:
                prod = tpool.tile([128, 512], FP32, tag="dummy")
                nc.vector.tensor_mul(prod[:], h[:], dwb_sb[:])
                nc.vector.tensor_reduce(dense_to, prod[:],
                                        mybir.AxisListType.X, ALU.add)
            return h

        def lstm_transpose(h, hT):
            trp = trpool.tile([128, 512], FP32, tag="tr")
            for k in range(4):
                nc.tensor.transpose(trp[:, k * 128:(k + 1) * 128],
                                    h[:, k * 128:(k + 1) * 128], id_sb[:])
            nc.vector.tensor_copy(hT[:], trp[:])

        def h_lhs(hT):
            return lambda k: hT[:, k * 128:(k + 1) * 128]

        def w_rhs(w_sb):
            return lambda k, n: w_sb[:, k * GH + n * 512:k * GH + (n + 1) * 512]

        # Layers run with a 1-step skew so the tensor engine always has the
        # other layer's matmuls to chew on while one layer's elementwise
        # chain + state transpose completes (PE executes in program order).
        h1_prev = None
        for t in range(t_enc):
            h0_t = lstm_gates(
                [(lambda k, _t=t: xT_sb[:, _t * 128:(_t + 1) * 128],
                  lambda k, n: we0_sb[:, n * 512:(n + 1) * 512], 1),
                 (h_lhs(h0T), w_rhs(ue0_sb), NKH)], c0)
            if t > 0:
                h1_prev = lstm_gates(
                    [(h_lhs(h0T), w_rhs(we1_sb), NKH),
                     (h_lhs(h1T), w_rhs(ue1_sb), NKH)], c1)
            lstm_transpose(h0_t, h0T)
            if t > 0:
                lstm_transpose(h1_prev, h1T)
        h1_last = lstm_gates(
            [(h_lhs(h0T), w_rhs(we1_sb), NKH),
             (h_lhs(h1T), w_rhs(ue1_sb), NKH)], c1)
        lstm_transpose(h1_last, h1T)

        # decoder weights reuse the encoder weight slots (tag sharing)
        ud0_sb = load(u_d0, NKH * GH, "u0", 8)
        wd1_sb = load(w_d1, NKH * GH, "w1", 8)
        ud1_sb = load(u_d1, NKH * GH, "u1", 8)

        hd1_prev = None
        for t in range(seg):
            hd0_t = lstm_gates([(h_lhs(h0T), w_rhs(ud0_sb), NKH)], c0)
            if t > 0:
                hd1_prev = lstm_gates(
                    [(h_lhs(h0T), w_rhs(wd1_sb), NKH),
                     (h_lhs(h1T), w_rhs(ud1_sb), NKH)], c1,
                    dense_to=out_sb[:, t - 1:t])
            lstm_transpose(hd0_t, h0T)
            if t > 0:
                lstm_transpose(hd1_prev, h1T)
        lstm_gates(
            [(h_lhs(h0T), w_rhs(wd1_sb), NKH),
             (h_lhs(h1T), w_rhs(ud1_sb), NKH)], c1,
            dense_to=out_sb[:, seg - 1:seg])

        nc.sync.dma_start(out[:], out_sb[:])

    nc.compile()
    return nc


def _make_callable(nc):
    import jax
    from jax.sharding import Mesh, PartitionSpec
    from jax.experimental.shard_map import shard_map
    from concourse.bass2jax import (_bass_exec_p, install_neuronx_cc_hook,
                                    partition_id_tensor)

    install_neuronx_cc_hook()
    partition_name = (nc.partition_id_tensor.name
                      if nc.partition_id_tensor else None)
    in_names, out_names, out_avals = [], [], []
    for alloc in nc.m.functions[0].allocations:
        if not isinstance(alloc, mybir.MemoryLocationSet):
            continue
        name = alloc.memorylocations[0].name
        if alloc.kind == "ExternalInput":
            if name != partition_name:
                in_names.append(name)
        elif alloc.kind == "ExternalOutput":
            out_names.append(name)
            out_avals.append(jax.core.ShapedArray(
                tuple(alloc.tensor_shape), mybir.dt.np(alloc.dtype)))
    n_params = len(in_names)
    in_names_all = list(in_names) + list(out_names)
    if partition_name is not None:
        in_names_all.append(partition_name)

    def _body(*args):
        operands = list(args)
        if partition_name is not None:
            operands.append(partition_id_tensor())
        return tuple(_bass_exec_p.bind(
            *operands, out_avals=tuple(out_avals), in_names=tuple(in_names_all),
            out_names=tuple(out_names), lowering_input_output_aliases=(),
            sim_require_finite=True, sim_require_nnan=True, nc=nc))

    devices = jax.devices()[:N_CORES]
    mesh = Mesh(np.asarray(devices), ("core",))
    n_outs = len(out_names)
    sharded = jax.jit(
        shard_map(_body, mesh=mesh,
                  in_specs=(PartitionSpec("core"),) * (n_params + n_outs),
                  out_specs=(PartitionSpec("core"),) * n_outs,
                  check_rep=False),
        donate_argnums=tuple(range(n_params, n_params + n_outs)),
        keep_unused=True)
    return sharded, in_names, out_names, out_avals


def _prep_w(w, nk, bias=None):
    """[K, GH] weight -> [128, nk*GH] tile layout; optional bias folded into
    the first zero-pad row (requires K < nk*128)."""
    w = np.asarray(w, np.float32)
    k_in = w.shape[0]
    wp = np.zeros((nk * 128, GH), np.float32)
    wp[:k_in] = w
    if bias is not None:
        wp[k_in] = np.asarray(bias, np.float32)
    return np.ascontiguousarray(
        wp.reshape(nk, 128, GH).transpose(1, 0, 2).reshape(128, nk * GH))


def _get_runtime(t_enc, seg):
    key = (t_enc, seg)
    if key not in _RUNTIME:
        nc = _build_program(t_enc, seg)
        _RUNTIME[key] = _make_callable(nc)
    return _RUNTIME[key]


def _run(in_maps, t_enc, seg):
    import jax
    fn, in_names, out_names, out_avals = _get_runtime(t_enc, seg)
    per_core = [[np.asarray(m[name]) for name in in_names] for m in in_maps]
    concat_in = [np.concatenate([per_core[c][i] for c in range(N_CORES)], axis=0)
                 for i in range(len(in_names))]
    concat_zeros = [np.zeros((N_CORES * a.shape[0], *a.shape[1:]), a.dtype)
                    for a in out_avals]
    outs = fn(*concat_in, *concat_zeros)
    outs = [np.asarray(o) for o in outs]
    return [{name: outs[i].reshape(N_CORES, *out_avals[i].shape)[c]
             for i, name in enumerate(out_names)}
            for c in range(N_CORES)]


def _numpy_ref(x, dec_in, eW0, eU0, eb0, eW1, eU1, eb1,
               dW0, dU0, db0, dW1, dU1, db1, denseW, denseb):
    def sig(v):
        return 1.0 / (1.0 + np.exp(-v))

    def scan(xs, h, c, W, U, b):
        ys = []
        for t in range(xs.shape[1]):
            z = xs[:, t] @ W + h @ U + b
            i, f, g, o = np.split(z, 4, axis=-1)
            c = sig(f) * c + sig(i) * np.tanh(g)
            h = sig(o) * np.tanh(c)
            ys.append(h)
        return np.stack(ys, 1), h, c

    b = x.shape[0]
    z = np.zeros((b, H), np.float32)
    y0, h0, c0 = scan(x, z, z, eW0, eU0, eb0)
    _, h1, c1 = scan(y0, z, z, eW1, eU1, eb1)
    d0, _, _ = scan(dec_in, h0, c0, dW0, dU0, db0)
    d1, _, _ = scan(d0, h1, c1, dW1, dU1, db1)
    return (d1 @ denseW + denseb).astype(np.float32)


def make_in_maps(x, eW0, eU0, eb0, eW1, eU1, dU0, dW1, dU1, denseW,
                 t_enc):
    x = np.asarray(x, np.float32)
    shared = {
        "w_e0": _prep_w(np.asarray(eW0, np.float32), 1, bias=eb0),
        "u_e0": _prep_w(eU0, NKH),
        "w_e1": _prep_w(eW1, NKH),
        "u_e1": _prep_w(eU1, NKH),
        "u_d0": _prep_w(dU0, NKH),
        "w_d1": _prep_w(dW1, NKH),
        "u_d1": _prep_w(dU1, NKH),
        "ident": np.eye(128, dtype=np.float32),
        "dwb": np.ascontiguousarray(
            np.tile(np.asarray(denseW, np.float32).reshape(1, H), (128, 1))),
    }
    in_maps = []
    for c in range(N_CORES):
        xs = x[c * BL:(c + 1) * BL]                       # [128, t, F]
        xt = np.zeros((128, t_enc * 128), np.float32)
        xt[:F] = xs.transpose(2, 1, 0).reshape(F, -1)
        xt[F] = 1.0                                        # bias ones-row
        in_maps.append({"xT": np.ascontiguousarray(xt), **shared})
    return in_maps


def kernel(x, dec_in, eW0, eU0, eb0, eW1, eU1, eb1,
           dW0, dU0, db0, dW1, dU1, db1, denseW, denseb):
    x = np.asarray(x, np.float32)
    dec_in = np.asarray(dec_in, np.float32)
    # Generic-input guard: the on-device fast path folds eb0 and assumes the
    # remaining biases and dec_in are zero (true for this model's inputs).
    if (np.any(dec_in) or np.any(np.asarray(eb1)) or np.any(np.asarray(db0))
            or np.any(np.asarray(db1))):
        return _numpy_ref(x, dec_in, np.asarray(eW0), np.asarray(eU0),
                          np.asarray(eb0), np.asarray(eW1), np.asarray(eU1),
                          np.asarray(eb1), np.asarray(dW0), np.asarray(dU0),
                          np.asarray(db0), np.asarray(dW1), np.asarray(dU1),
                          np.asarray(db1), np.asarray(denseW),
                          np.asarray(denseb))

    t_enc, seg = x.shape[1], dec_in.shape[1]
    in_maps = make_in_maps(x, eW0, eU0, eb0, eW1, eU1, dU0, dW1, dU1,
                           denseW, t_enc)
    results = _run(in_maps, t_enc, seg)
    out = np.concatenate([results[c]["out"] for c in range(N_CORES)], axis=0)
    out = out + np.asarray(denseb, np.float32).reshape(1, 1)
    return out.reshape(B, seg, 1).astype(np.float32)



# revision 9
# speedup vs baseline: 1.1870x; 1.1870x over previous
"""Trainium2 Bass kernel for nn_KerasSeq2Seq: 2-layer LSTM encoder (T=64) +
2-layer LSTM decoder (SEG=32) + Dense(1), B=1024, H=512, F=121.

Sharding: data-parallel over batch across 8 NeuronCores (128 rows each),
weights replicated. Per core, per step, gate pre-activations are computed as
PSUM-accumulated bf16 matmuls (1 cycle/row on the PE vs 4 for fp32) with the
*transposed* hidden state as the stationary operand; hidden states are
re-transposed each step on the tensor engine. Cell state c stays fp32.
"""

import sys
from contextlib import ExitStack

import numpy as np

sys.path.insert(0, "/opt/trn_rl_repo")

import concourse.bass as bass  # noqa: E402
import concourse.tile as tile  # noqa: E402
from concourse import bacc, mybir  # noqa: E402

N_CORES = 8
B, T_ENC, F, H, SEG = 1024, 64, 121, 512, 32
BL = B // N_CORES            # 128 batch rows per core
GH = 4 * H                   # 2048 gate columns
NKH = H // 128               # 4 K-chunks for an H-dim contraction
FP32 = mybir.dt.float32
BF16 = mybir.dt.bfloat16
AF = mybir.ActivationFunctionType
ALU = mybir.AluOpType

_RUNTIME = {}


def _bf16(a):
    import ml_dtypes
    return np.asarray(a, np.float32).astype(ml_dtypes.bfloat16)


def _build_program(t_enc, seg, reps=1):
    """reps>1 replicates the whole computation (including weight/input DMA)
    back-to-back in one program — used by test.py to time the kernel on
    hardware via the slope between reps=1 and reps=N wall times (the per-call
    RPC overhead cancels exactly)."""
    nc = bacc.Bacc("TRN2", target_bir_lowering=False, debug=False,
                   num_devices=N_CORES)

    xT = nc.dram_tensor("xT", [128, t_enc * 128], BF16, kind="ExternalInput").ap()
    w_e0 = nc.dram_tensor("w_e0", [128, GH], BF16, kind="ExternalInput").ap()
    u_e0 = nc.dram_tensor("u_e0", [128, NKH * GH], BF16, kind="ExternalInput").ap()
    w_e1 = nc.dram_tensor("w_e1", [128, NKH * GH], BF16, kind="ExternalInput").ap()
    u_e1 = nc.dram_tensor("u_e1", [128, NKH * GH], BF16, kind="ExternalInput").ap()
    u_d0 = nc.dram_tensor("u_d0", [128, NKH * GH], BF16, kind="ExternalInput").ap()
    w_d1 = nc.dram_tensor("w_d1", [128, NKH * GH], BF16, kind="ExternalInput").ap()
    u_d1 = nc.dram_tensor("u_d1", [128, NKH * GH], BF16, kind="ExternalInput").ap()
    ident = nc.dram_tensor("ident", [128, 128], BF16, kind="ExternalInput").ap()
    dwb = nc.dram_tensor("dwb", [128, H], BF16, kind="ExternalInput").ap()
    out = nc.dram_tensor("out", [128, seg], FP32, kind="ExternalOutput").ap()

    with tile.TileContext(nc) as tc, ExitStack() as ctx:
        wpool = ctx.enter_context(tc.tile_pool(name="w", bufs=1))
        zpool = ctx.enter_context(
            tc.tile_pool(name="z", bufs=6, space=bass.MemorySpace.PSUM))
        trpool = ctx.enter_context(
            tc.tile_pool(name="tr", bufs=2, space=bass.MemorySpace.PSUM))
        gpool = ctx.enter_context(tc.tile_pool(name="g", bufs=8))
        tpool = ctx.enter_context(tc.tile_pool(name="tmp", bufs=3))
        spool = ctx.enter_context(tc.tile_pool(name="state", bufs=1))

        for _rep in range(reps):
            _body_once(nc, tc, wpool, zpool, trpool, gpool, tpool, spool,
                       xT, w_e0, u_e0, w_e1, u_e1, u_d0, w_d1, u_d1, ident,
                       dwb, out, t_enc, seg)

    nc.compile()
    return nc


def _body_once(nc, tc, wpool, zpool, trpool, gpool, tpool, spool,
               xT, w_e0, u_e0, w_e1, u_e1, u_d0, w_d1, u_d1, ident,
               dwb, out, t_enc, seg):
    if True:
        def load(dram_ap, cols, tag, nsplit):
            t = wpool.tile([128, cols], BF16, tag=tag)
            w = cols // nsplit
            for i in range(nsplit):
                nc.sync.dma_start(t[:, i * w:(i + 1) * w],
                                  dram_ap[:, i * w:(i + 1) * w])
            return t

        # all weights fit in SBUF as bf16. DMA issue order matters: step 0
        # only needs xT's first quarter + w_e0 (the h@U matmuls are skipped
        # at t=0 since h==0), so those go first and compute starts almost
        # immediately while the rest streams in behind.
        xT_sb = load(xT, t_enc * 128, "xT", min(4, t_enc))
        we0_sb = load(w_e0, GH, "we0", 2)
        id_sb = wpool.tile([128, 128], BF16, tag="ident")
        nc.sync.dma_start(id_sb[:], ident[:])
        ue0_sb = load(u_e0, NKH * GH, "u0", 8)
        we1_sb = load(w_e1, NKH * GH, "w1", 8)
        ue1_sb = load(u_e1, NKH * GH, "u1", 8)
        ud0_sb = load(u_d0, NKH * GH, "ud0", 8)
        wd1_sb = load(w_d1, NKH * GH, "wd1", 8)
        ud1_sb = load(u_d1, NKH * GH, "ud1", 8)
        dwb_sb = wpool.tile([128, H], BF16, tag="dwb")
        nc.sync.dma_start(dwb_sb[:], dwb[:])

        h0T = spool.tile([128, H], BF16, tag="h0T")
        h1T = spool.tile([128, H], BF16, tag="h1T")
        c0 = spool.tile([128, H], FP32, tag="c0")
        c1 = spool.tile([128, H], FP32, tag="c1")
        out_sb = spool.tile([128, seg], FP32, tag="out")

        def lstm_gates(ins, c, dense_to=None, first=False):
            """Matmuls + activations + c/h update. Returns the h tile.
            ins: list of (lhs_fn(k) -> AP[128,128], rhs_fn(k, n) -> AP[128,512], kc)
            k-outer / gate-inner so 4 consecutive matmuls share the stationary
            operand (one LDWEIGHTS per k-chunk). first=True means h==c==0 on
            entry: caller omits the h@U groups and c is written as i*g."""
            tot = sum(kc for _, _, kc in ins)
            zs = [zpool.tile([128, 512], FP32, tag="z", name=f"z{n}")
                  for n in range(4)]
            cnt = 0
            for (lhs_fn, rhs_fn, kc) in ins:
                for k in range(kc):
                    cnt += 1
                    for n in range(4):
                        nc.tensor.matmul(zs[n][:], lhs_fn(k), rhs_fn(k, n),
                                         start=(cnt == 1), stop=(cnt == tot))
            gates = []
            for n in range(4):
                g_t = gpool.tile([128, 512], BF16, tag="gate")
                nc.scalar.activation(g_t[:], zs[n][:],
                                     AF.Tanh if n == 2 else AF.Sigmoid)
                gates.append(g_t)
            i_t, f_t, g_t, o_t = gates
            if first:
                nc.vector.tensor_mul(c[:], i_t[:], g_t[:])
            else:
                ig = tpool.tile([128, 512], FP32, tag="ig")
                nc.vector.tensor_mul(ig[:], i_t[:], g_t[:])
                nc.vector.tensor_mul(c[:], f_t[:], c[:])
                nc.vector.tensor_add(c[:], c[:], ig[:])
            tc_t = tpool.tile([128, 512], BF16, tag="tc")
            nc.scalar.activation(tc_t[:], c[:], AF.Tanh)
            h = tpool.tile([128, 512], BF16, tag="h")
            nc.vector.tensor_mul(h[:], o_t[:], tc_t[:])
            if dense_to is not None:
                prod = tpool.tile([128, 512], FP32, tag="dummy")
                nc.vector.tensor_mul(prod[:], h[:], dwb_sb[:])
                nc.vector.tensor_reduce(dense_to, prod[:],
                                        mybir.AxisListType.X, ALU.add)
            return h

        def lstm_transpose(h, hT):
            # per-chunk PSUM->SBUF copies so the next step's k-chunk matmuls
            # (stationary operand = hT chunk k) can start as each chunk lands
            trp = trpool.tile([128, 512], BF16, tag="tr")
            for k in range(4):
                nc.tensor.transpose(trp[:, k * 128:(k + 1) * 128],
                                    h[:, k * 128:(k + 1) * 128], id_sb[:])
            for k in range(4):
                nc.vector.tensor_copy(hT[:, k * 128:(k + 1) * 128],
                                      trp[:, k * 128:(k + 1) * 128])

        def h_lhs(hT):
            return lambda k: hT[:, k * 128:(k + 1) * 128]

        def w_rhs(w_sb):
            return lambda k, n: w_sb[:, k * GH + n * 512:k * GH + (n + 1) * 512]

        # Layers run with a 1-step skew so the tensor engine always has the
        # other layer's matmuls to chew on while one layer's elementwise
        # chain + state transpose completes (PE executes in program order).
        h1_prev = None
        for t in range(t_enc):
            xgroup = (lambda k, _t=t: xT_sb[:, _t * 128:(_t + 1) * 128],
                      lambda k, n: we0_sb[:, n * 512:(n + 1) * 512], 1)
            if t == 0:
                h0_t = lstm_gates([xgroup], c0, first=True)
            else:
                h0_t = lstm_gates(
                    [xgroup, (h_lhs(h0T), w_rhs(ue0_sb), NKH)], c0)
            if t == 1:
                h1_prev = lstm_gates(
                    [(h_lhs(h0T), w_rhs(we1_sb), NKH)], c1, first=True)
            elif t > 1:
                h1_prev = lstm_gates(
                    [(h_lhs(h0T), w_rhs(we1_sb), NKH),
                     (h_lhs(h1T), w_rhs(ue1_sb), NKH)], c1)
            lstm_transpose(h0_t, h0T)
            if t > 0:
                lstm_transpose(h1_prev, h1T)
        h1_last = lstm_gates(
            [(h_lhs(h0T), w_rhs(we1_sb), NKH),
             (h_lhs(h1T), w_rhs(ue1_sb), NKH)], c1)
        lstm_transpose(h1_last, h1T)

        hd1_prev = None
        for t in range(seg):
            hd0_t = lstm_gates([(h_lhs(h0T), w_rhs(ud0_sb), NKH)], c0)
            if t > 0:
                hd1_prev = lstm_gates(
                    [(h_lhs(h0T), w_rhs(wd1_sb), NKH),
                     (h_lhs(h1T), w_rhs(ud1_sb), NKH)], c1,
                    dense_to=out_sb[:, t - 1:t])
            lstm_transpose(hd0_t, h0T)
            if t > 0:
                lstm_transpose(hd1_prev, h1T)
        lstm_gates(
            [(h_lhs(h0T), w_rhs(wd1_sb), NKH),
             (h_lhs(h1T), w_rhs(ud1_sb), NKH)], c1,
            dense_to=out_sb[:, seg - 1:seg])

        nc.sync.dma_start(out[:], out_sb[:])


def _make_callable(nc):
    import jax
    from jax.sharding import Mesh, PartitionSpec
    from jax.experimental.shard_map import shard_map
    from concourse.bass2jax import (_bass_exec_p, install_neuronx_cc_hook,
                                    partition_id_tensor)

    install_neuronx_cc_hook()
    partition_name = (nc.partition_id_tensor.name
                      if nc.partition_id_tensor else None)
    in_names, out_names, out_avals = [], [], []
    for alloc in nc.m.functions[0].allocations:
        if not isinstance(alloc, mybir.MemoryLocationSet):
            continue
        name = alloc.memorylocations[0].name
        if alloc.kind == "ExternalInput":
            if name != partition_name:
                in_names.append(name)
        elif alloc.kind == "ExternalOutput":
            out_names.append(name)
            out_avals.append(jax.core.ShapedArray(
                tuple(alloc.tensor_shape), mybir.dt.np(alloc.dtype)))
    n_params = len(in_names)
    in_names_all = list(in_names) + list(out_names)
    if partition_name is not None:
        in_names_all.append(partition_name)

    def _body(*args):
        operands = list(args)
        if partition_name is not None:
            operands.append(partition_id_tensor())
        return tuple(_bass_exec_p.bind(
            *operands, out_avals=tuple(out_avals), in_names=tuple(in_names_all),
            out_names=tuple(out_names), lowering_input_output_aliases=(),
            sim_require_finite=True, sim_require_nnan=True, nc=nc))

    devices = jax.devices()[:N_CORES]
    mesh = Mesh(np.asarray(devices), ("core",))
    n_outs = len(out_names)
    sharded = jax.jit(
        shard_map(_body, mesh=mesh,
                  in_specs=(PartitionSpec("core"),) * (n_params + n_outs),
                  out_specs=(PartitionSpec("core"),) * n_outs,
                  check_rep=False),
        donate_argnums=tuple(range(n_params, n_params + n_outs)),
        keep_unused=True)
    return sharded, in_names, out_names, out_avals


def _prep_w(w, nk, bias=None):
    """[K, GH] weight -> [128, nk*GH] bf16 tile layout; optional bias folded
    into the first zero-pad row (requires K < nk*128)."""
    w = np.asarray(w, np.float32)
    k_in = w.shape[0]
    wp = np.zeros((nk * 128, GH), np.float32)
    wp[:k_in] = w
    if bias is not None:
        wp[k_in] = np.asarray(bias, np.float32)
    return _bf16(np.ascontiguousarray(
        wp.reshape(nk, 128, GH).transpose(1, 0, 2).reshape(128, nk * GH)))


def _get_runtime(t_enc, seg):
    key = (t_enc, seg)
    if key not in _RUNTIME:
        nc = _build_program(t_enc, seg)
        _RUNTIME[key] = _make_callable(nc)
    return _RUNTIME[key]


def _run(in_maps, t_enc, seg):
    import jax
    fn, in_names, out_names, out_avals = _get_runtime(t_enc, seg)
    per_core = [[np.asarray(m[name]) for name in in_names] for m in in_maps]
    concat_in = [np.concatenate([per_core[c][i] for c in range(N_CORES)], axis=0)
                 for i in range(len(in_names))]
    concat_zeros = [np.zeros((N_CORES * a.shape[0], *a.shape[1:]), a.dtype)
                    for a in out_avals]
    outs = fn(*concat_in, *concat_zeros)
    outs = [np.asarray(o) for o in outs]
    return [{name: outs[i].reshape(N_CORES, *out_avals[i].shape)[c]
             for i, name in enumerate(out_names)}
            for c in range(N_CORES)]


def _numpy_ref(x, dec_in, eW0, eU0, eb0, eW1, eU1, eb1,
               dW0, dU0, db0, dW1, dU1, db1, denseW, denseb):
    def sig(v):
        return 1.0 / (1.0 + np.exp(-v))

    def scan(xs, h, c, W, U, b):
        ys = []
        for t in range(xs.shape[1]):
            z = xs[:, t] @ W + h @ U + b
            i, f, g, o = np.split(z, 4, axis=-1)
            c = sig(f) * c + sig(i) * np.tanh(g)
            h = sig(o) * np.tanh(c)
            ys.append(h)
        return np.stack(ys, 1), h, c

    b = x.shape[0]
    z = np.zeros((b, H), np.float32)
    y0, h0, c0 = scan(x, z, z, eW0, eU0, eb0)
    _, h1, c1 = scan(y0, z, z, eW1, eU1, eb1)
    d0, _, _ = scan(dec_in, h0, c0, dW0, dU0, db0)
    d1, _, _ = scan(d0, h1, c1, dW1, dU1, db1)
    return (d1 @ denseW + denseb).astype(np.float32)


def make_in_maps(x, eW0, eU0, eb0, eW1, eU1, dU0, dW1, dU1, denseW,
                 t_enc):
    x = np.asarray(x, np.float32)
    shared = {
        "w_e0": _prep_w(np.asarray(eW0, np.float32), 1, bias=eb0),
        "u_e0": _prep_w(eU0, NKH),
        "w_e1": _prep_w(eW1, NKH),
        "u_e1": _prep_w(eU1, NKH),
        "u_d0": _prep_w(dU0, NKH),
        "w_d1": _prep_w(dW1, NKH),
        "u_d1": _prep_w(dU1, NKH),
        "ident": _bf16(np.eye(128, dtype=np.float32)),
        "dwb": _bf16(np.ascontiguousarray(
            np.tile(np.asarray(denseW, np.float32).reshape(1, H), (128, 1)))),
    }
    in_maps = []
    for c in range(N_CORES):
        xs = x[c * BL:(c + 1) * BL]                       # [128, t, F]
        xt = np.zeros((128, t_enc * 128), np.float32)
        xt[:F] = xs.transpose(2, 1, 0).reshape(F, -1)
        xt[F] = 1.0                                        # bias ones-row
        in_maps.append({"xT": _bf16(xt), **shared})
    return in_maps


def kernel(x, dec_in, eW0, eU0, eb0, eW1, eU1, eb1,
           dW0, dU0, db0, dW1, dU1, db1, denseW, denseb):
    x = np.asarray(x, np.float32)
    dec_in = np.asarray(dec_in, np.float32)
    # Generic-input guard: the on-device fast path folds eb0 and assumes the
    # remaining biases and dec_in are zero (true for this model's inputs).
    if (np.any(dec_in) or np.any(np.asarray(eb1)) or np.any(np.asarray(db0))
            or np.any(np.asarray(db1))):
        return _numpy_ref(x, dec_in, np.asarray(eW0), np.asarray(eU0),
                          np.asarray(eb0), np.asarray(eW1), np.asarray(eU1),
                          np.asarray(eb1), np.asarray(dW0), np.asarray(dU0),
                          np.asarray(db0), np.asarray(dW1), np.asarray(dU1),
                          np.asarray(db1), np.asarray(denseW),
                          np.asarray(denseb))

    t_enc, seg = x.shape[1], dec_in.shape[1]
    in_maps = make_in_maps(x, eW0, eU0, eb0, eW1, eU1, dU0, dW1, dU1,
                           denseW, t_enc)
    results = _run(in_maps, t_enc, seg)
    out = np.concatenate([results[c]["out"] for c in range(N_CORES)], axis=0)
    out = out + np.asarray(denseb, np.float32).reshape(1, 1)
    return out.reshape(B, seg, 1).astype(np.float32)


# revision 25
# speedup vs baseline: 19.6616x; 16.5644x over previous
"""Trainium2 Bass kernel for nn_KerasSeq2Seq: 2-layer LSTM encoder (T=64) +
2-layer LSTM decoder (SEG=32) + Dense(1), B=1024, H=512, F=121.

Sharding: data-parallel over batch across 8 NeuronCores (128 rows each),
weights replicated. Per core, per step, gate pre-activations are computed as
PSUM-accumulated bf16 matmuls (1 cycle/row on the PE vs 4 for fp32) with the
*transposed* hidden state as the stationary operand; hidden states are
re-transposed each step on the tensor engine. Cell state c stays fp32.
"""

import sys
from contextlib import ExitStack

import numpy as np

sys.path.insert(0, "/opt/trn_rl_repo")

import concourse.bass as bass  # noqa: E402
import concourse.tile as tile  # noqa: E402
from concourse import bacc, mybir  # noqa: E402

N_CORES = 8
B, T_ENC, F, H, SEG = 1024, 64, 121, 512, 32
BL = B // N_CORES            # 128 batch rows per core
GH = 4 * H                   # 2048 gate columns
NKH = H // 128               # 4 K-chunks for an H-dim contraction
FP32 = mybir.dt.float32
BF16 = mybir.dt.bfloat16
AF = mybir.ActivationFunctionType
ALU = mybir.AluOpType

_RUNTIME = {}


def _bf16(a):
    import ml_dtypes
    return np.asarray(a, np.float32).astype(ml_dtypes.bfloat16)


def _build_program(t_enc, seg, reps=1, mode="full"):
    """reps>1 replicates the whole computation (including weight/input DMA)
    back-to-back in one program — used by test.py to time the kernel on
    hardware via the slope between reps=1 and reps=N wall times (the per-call
    RPC overhead cancels exactly). mode != "full" builds stripped variants
    (wrong numerics) used only for attributing time to kernel pieces."""
    nc = bacc.Bacc("TRN2", target_bir_lowering=False, debug=False,
                   num_devices=N_CORES)

    xT = nc.dram_tensor("xT", [128, t_enc * 128], BF16, kind="ExternalInput").ap()
    w_e0 = nc.dram_tensor("w_e0", [128, GH], BF16, kind="ExternalInput").ap()
    u_e0 = nc.dram_tensor("u_e0", [128, NKH * GH], BF16, kind="ExternalInput").ap()
    w_e1 = nc.dram_tensor("w_e1", [128, NKH * GH], BF16, kind="ExternalInput").ap()
    u_e1 = nc.dram_tensor("u_e1", [128, NKH * GH], BF16, kind="ExternalInput").ap()
    u_d0 = nc.dram_tensor("u_d0", [128, NKH * GH], BF16, kind="ExternalInput").ap()
    w_d1 = nc.dram_tensor("w_d1", [128, NKH * GH], BF16, kind="ExternalInput").ap()
    u_d1 = nc.dram_tensor("u_d1", [128, NKH * GH], BF16, kind="ExternalInput").ap()
    ident = nc.dram_tensor("ident", [128, 128], BF16, kind="ExternalInput").ap()
    dwb = nc.dram_tensor("dwb", [128, H], BF16, kind="ExternalInput").ap()
    out = nc.dram_tensor("out", [128, seg], FP32, kind="ExternalOutput").ap()

    with tile.TileContext(nc) as tc, ExitStack() as ctx:
        wpool = ctx.enter_context(tc.tile_pool(name="w", bufs=1))
        # PSUM budget (8 banks): sigmoid banks [i f o] x2 cells in flight (6)
        # + one shared tanh-gate bank + one shared transpose bank
        zpool = ctx.enter_context(
            tc.tile_pool(name="z", bufs=2, space=bass.MemorySpace.PSUM))
        zgpool = ctx.enter_context(
            tc.tile_pool(name="zg", bufs=1, space=bass.MemorySpace.PSUM))
        trpool = ctx.enter_context(
            tc.tile_pool(name="tr", bufs=1, space=bass.MemorySpace.PSUM))
        gpool = ctx.enter_context(tc.tile_pool(name="g", bufs=4))
        tpool = ctx.enter_context(tc.tile_pool(name="tmp", bufs=3))
        spool = ctx.enter_context(tc.tile_pool(name="state", bufs=1))

        for _rep in range(reps):
            _body_once(nc, tc, wpool, zpool, zgpool, trpool, gpool, tpool,
                       spool, xT, w_e0, u_e0, w_e1, u_e1, u_d0, w_d1, u_d1,
                       ident, dwb, out, t_enc, seg, mode)

    nc.compile()
    return nc


def _body_once(nc, tc, wpool, zpool, zgpool, trpool, gpool, tpool,
               spool, xT, w_e0, u_e0, w_e1, u_e1, u_d0, w_d1, u_d1,
               ident, dwb, out, t_enc, seg, mode="full"):
    if True:
        def load(dram_ap, cols, tag, nsplit):
            t = wpool.tile([128, cols], BF16, tag=tag)
            w = cols // nsplit
            for i in range(nsplit):
                nc.sync.dma_start(t[:, i * w:(i + 1) * w],
                                  dram_ap[:, i * w:(i + 1) * w])
            return t

        # all weights fit in SBUF as bf16. DMA issue order matters: step 0
        # only needs xT's first quarter + w_e0 (the h@U matmuls are skipped
        # at t=0 since h==0), so those go first and compute starts almost
        # immediately while the rest streams in behind.
        xT_sb = load(xT, t_enc * 128, "xT", min(4, t_enc))
        we0_sb = load(w_e0, GH, "we0", 2)
        id_sb = wpool.tile([128, 128], BF16, tag="ident")
        nc.sync.dma_start(id_sb[:], ident[:])
        ue0_sb = load(u_e0, NKH * GH, "u0", 8)
        we1_sb = load(w_e1, NKH * GH, "w1", 8)
        ue1_sb = load(u_e1, NKH * GH, "u1", 8)
        ud0_sb = load(u_d0, NKH * GH, "ud0", 8)
        wd1_sb = load(w_d1, NKH * GH, "wd1", 8)
        ud1_sb = load(u_d1, NKH * GH, "ud1", 8)
        dwb_sb = wpool.tile([128, H], BF16, tag="dwb")
        nc.sync.dma_start(dwb_sb[:], dwb[:])

        h0T = spool.tile([128, H], BF16, tag="h0T")
        h1T = spool.tile([128, H], BF16, tag="h1T")
        c0 = spool.tile([128, H], FP32, tag="c0")
        c1 = spool.tile([128, H], FP32, tag="c1")
        out_sb = spool.tile([128, seg], FP32, tag="out")
        if mode in ("mm", "mmact"):
            nc.vector.memset(out_sb[:], 0.0)
        if mode in ("mm", "mmact", "notrans"):
            for s in (h0T, h1T, c0, c1):
                nc.vector.memset(s[:], 0.0)

        def lstm_gates(ins, c, dense_to=None, first=False):
            """Matmuls + activations + c/h update. Returns the h tile.
            ins: list of (lhs_fn(k) -> AP[128,128], rhs_fn(k, n) -> AP[128,512], kc)
            k-outer / gate-inner so 4 consecutive matmuls share the stationary
            operand (one LDWEIGHTS per k-chunk). Weight columns are pre-permuted
            host-side to [i f o g] so one sigmoid covers banks 0-2 and one tanh
            bank 3. first=True means h==c==0 on entry: caller omits the h@U
            groups and c is written as i*g."""
            tot = sum(kc for _, _, kc in ins)
            zs = zpool.tile([128, 1536], FP32, tag="z")
            zg = zgpool.tile([128, 512], FP32, tag="zg")
            cnt = 0
            for (lhs_fn, rhs_fn, kc) in ins:
                for k in range(kc):
                    cnt += 1
                    for n in range(3):
                        nc.tensor.matmul(zs[:, n * 512:(n + 1) * 512],
                                         lhs_fn(k), rhs_fn(k, n),
                                         start=(cnt == 1), stop=(cnt == tot))
            cnt = 0
            for (lhs_fn, rhs_fn, kc) in ins:
                for k in range(kc):
                    cnt += 1
                    nc.tensor.matmul(zg[:], lhs_fn(k), rhs_fn(k, 3),
                                     start=(cnt == 1), stop=(cnt == tot))
            if mode == "mm":
                return None
            sig = gpool.tile([128, 1536], BF16, tag="sig")
            nc.scalar.activation(sig[:], zs[:], AF.Sigmoid)
            g_t = gpool.tile([128, 512], BF16, tag="gate")
            nc.scalar.activation(g_t[:], zg[:], AF.Tanh)
            if mode == "mmact":
                return None
            i_t, f_t, o_t = sig[:, 0:512], sig[:, 512:1024], sig[:, 1024:1536]
            if first:
                nc.vector.tensor_mul(c[:], i_t, g_t[:])
            else:
                ig = tpool.tile([128, 512], FP32, tag="ig")
                nc.vector.tensor_mul(ig[:], i_t, g_t[:])
                nc.vector.tensor_mul(c[:], f_t, c[:])
                nc.vector.tensor_add(c[:], c[:], ig[:])
            tc_t = tpool.tile([128, 512], BF16, tag="tc")
            nc.scalar.activation(tc_t[:], c[:], AF.Tanh)
            h = tpool.tile([128, 512], BF16, tag="h")
            nc.vector.tensor_mul(h[:], o_t, tc_t[:])
            if dense_to is not None:
                prod = tpool.tile([128, 512], FP32, tag="dummy")
                nc.vector.tensor_mul(prod[:], h[:], dwb_sb[:])
                nc.vector.tensor_reduce(dense_to, prod[:],
                                        mybir.AxisListType.X, ALU.add)
            return h

        def lstm_transpose(h, hT):
            if mode in ("mm", "mmact", "notrans"):
                return
            trp = trpool.tile([128, 512], BF16, tag="tr")
            for k in range(4):
                nc.tensor.transpose(trp[:, k * 128:(k + 1) * 128],
                                    h[:, k * 128:(k + 1) * 128], id_sb[:])
            nc.scalar.copy(hT[:], trp[:])

        def h_lhs(hT):
            return lambda k: hT[:, k * 128:(k + 1) * 128]

        def w_rhs(w_sb):
            return lambda k, n: w_sb[:, k * GH + n * 512:k * GH + (n + 1) * 512]

        # Layers run with a 1-step skew so the tensor engine always has the
        # other layer's matmuls to chew on while one layer's elementwise
        # chain + state transpose completes (PE executes in program order).
        h1_prev = None
        for t in range(t_enc):
            xgroup = (lambda k, _t=t: xT_sb[:, _t * 128:(_t + 1) * 128],
                      lambda k, n: we0_sb[:, n * 512:(n + 1) * 512], 1)
            if t == 0:
                h0_t = lstm_gates([xgroup], c0, first=True)
            else:
                h0_t = lstm_gates(
                    [xgroup, (h_lhs(h0T), w_rhs(ue0_sb), NKH)], c0)
            if t == 1:
                h1_prev = lstm_gates(
                    [(h_lhs(h0T), w_rhs(we1_sb), NKH)], c1, first=True)
            elif t > 1:
                h1_prev = lstm_gates(
                    [(h_lhs(h0T), w_rhs(we1_sb), NKH),
                     (h_lhs(h1T), w_rhs(ue1_sb), NKH)], c1)
            lstm_transpose(h0_t, h0T)
            if t > 0:
                lstm_transpose(h1_prev, h1T)
        h1_last = lstm_gates(
            [(h_lhs(h0T), w_rhs(we1_sb), NKH),
             (h_lhs(h1T), w_rhs(ue1_sb), NKH)], c1)
        lstm_transpose(h1_last, h1T)

        hd1_prev = None
        for t in range(seg):
            hd0_t = lstm_gates([(h_lhs(h0T), w_rhs(ud0_sb), NKH)], c0)
            if t > 0:
                hd1_prev = lstm_gates(
                    [(h_lhs(h0T), w_rhs(wd1_sb), NKH),
                     (h_lhs(h1T), w_rhs(ud1_sb), NKH)], c1,
                    dense_to=out_sb[:, t - 1:t])
            lstm_transpose(hd0_t, h0T)
            if t > 0:
                lstm_transpose(hd1_prev, h1T)
        lstm_gates(
            [(h_lhs(h0T), w_rhs(wd1_sb), NKH),
             (h_lhs(h1T), w_rhs(ud1_sb), NKH)], c1,
            dense_to=out_sb[:, seg - 1:seg])

        nc.sync.dma_start(out[:], out_sb[:])


def _make_callable(nc):
    import jax
    from jax.sharding import Mesh, PartitionSpec
    from jax.experimental.shard_map import shard_map
    from concourse.bass2jax import (_bass_exec_p, install_neuronx_cc_hook,
                                    partition_id_tensor)

    install_neuronx_cc_hook()
    partition_name = (nc.partition_id_tensor.name
                      if nc.partition_id_tensor else None)
    in_names, out_names, out_avals = [], [], []
    for alloc in nc.m.functions[0].allocations:
        if not isinstance(alloc, mybir.MemoryLocationSet):
            continue
        name = alloc.memorylocations[0].name
        if alloc.kind == "ExternalInput":
            if name != partition_name:
                in_names.append(name)
        elif alloc.kind == "ExternalOutput":
            out_names.append(name)
            out_avals.append(jax.core.ShapedArray(
                tuple(alloc.tensor_shape), mybir.dt.np(alloc.dtype)))
    n_params = len(in_names)
    in_names_all = list(in_names) + list(out_names)
    if partition_name is not None:
        in_names_all.append(partition_name)

    def _body(*args):
        operands = list(args)
        if partition_name is not None:
            operands.append(partition_id_tensor())
        return tuple(_bass_exec_p.bind(
            *operands, out_avals=tuple(out_avals), in_names=tuple(in_names_all),
            out_names=tuple(out_names), lowering_input_output_aliases=(),
            sim_require_finite=True, sim_require_nnan=True, nc=nc))

    devices = jax.devices()[:N_CORES]
    mesh = Mesh(np.asarray(devices), ("core",))
    n_outs = len(out_names)
    sharded = jax.jit(
        shard_map(_body, mesh=mesh,
                  in_specs=(PartitionSpec("core"),) * (n_params + n_outs),
                  out_specs=(PartitionSpec("core"),) * n_outs,
                  check_rep=False),
        donate_argnums=tuple(range(n_params, n_params + n_outs)),
        keep_unused=True)
    return sharded, in_names, out_names, out_avals


def _prep_w(w, nk, bias=None):
    """[K, GH] weight -> [128, nk*GH] bf16 tile layout; optional bias folded
    into the first zero-pad row (requires K < nk*128). Gate columns are
    permuted from Keras [i f g o] to [i f o g] so the kernel's single sigmoid
    covers a contiguous 1536-col span and tanh the last 512."""
    w = np.asarray(w, np.float32)
    k_in = w.shape[0]
    wp = np.zeros((nk * 128, GH), np.float32)
    wp[:k_in] = w
    if bias is not None:
        wp[k_in] = np.asarray(bias, np.float32)
    wp = np.concatenate([wp[:, 0:512], wp[:, 512:1024],
                         wp[:, 1536:2048], wp[:, 1024:1536]], axis=1)
    return _bf16(np.ascontiguousarray(
        wp.reshape(nk, 128, GH).transpose(1, 0, 2).reshape(128, nk * GH)))


def _get_runtime(t_enc, seg):
    key = (t_enc, seg)
    if key not in _RUNTIME:
        nc = _build_program(t_enc, seg)
        _RUNTIME[key] = _make_callable(nc)
    return _RUNTIME[key]


def _run(in_maps, t_enc, seg):
    import jax
    fn, in_names, out_names, out_avals = _get_runtime(t_enc, seg)
    per_core = [[np.asarray(m[name]) for name in in_names] for m in in_maps]
    concat_in = [np.concatenate([per_core[c][i] for c in range(N_CORES)], axis=0)
                 for i in range(len(in_names))]
    concat_zeros = [np.zeros((N_CORES * a.shape[0], *a.shape[1:]), a.dtype)
                    for a in out_avals]
    outs = fn(*concat_in, *concat_zeros)
    outs = [np.asarray(o) for o in outs]
    return [{name: outs[i].reshape(N_CORES, *out_avals[i].shape)[c]
             for i, name in enumerate(out_names)}
            for c in range(N_CORES)]


def _numpy_ref(x, dec_in, eW0, eU0, eb0, eW1, eU1, eb1,
               dW0, dU0, db0, dW1, dU1, db1, denseW, denseb):
    def sig(v):
        return 1.0 / (1.0 + np.exp(-v))

    def scan(xs, h, c, W, U, b):
        ys = []
        for t in range(xs.shape[1]):
            z = xs[:, t] @ W + h @ U + b
            i, f, g, o = np.split(z, 4, axis=-1)
            c = sig(f) * c + sig(i) * np.tanh(g)
            h = sig(o) * np.tanh(c)
            ys.append(h)
        return np.stack(ys, 1), h, c

    b = x.shape[0]
    z = np.zeros((b, H), np.float32)
    y0, h0, c0 = scan(x, z, z, eW0, eU0, eb0)
    _, h1, c1 = scan(y0, z, z, eW1, eU1, eb1)
    d0, _, _ = scan(dec_in, h0, c0, dW0, dU0, db0)
    d1, _, _ = scan(d0, h1, c1, dW1, dU1, db1)
    return (d1 @ denseW + denseb).astype(np.float32)


def make_in_maps(x, eW0, eU0, eb0, eW1, eU1, dU0, dW1, dU1, denseW,
                 t_enc):
    x = np.asarray(x, np.float32)
    shared = {
        "w_e0": _prep_w(np.asarray(eW0, np.float32), 1, bias=eb0),
        "u_e0": _prep_w(eU0, NKH),
        "w_e1": _prep_w(eW1, NKH),
        "u_e1": _prep_w(eU1, NKH),
        "u_d0": _prep_w(dU0, NKH),
        "w_d1": _prep_w(dW1, NKH),
        "u_d1": _prep_w(dU1, NKH),
        "ident": _bf16(np.eye(128, dtype=np.float32)),
        "dwb": _bf16(np.ascontiguousarray(
            np.tile(np.asarray(denseW, np.float32).reshape(1, H), (128, 1)))),
    }
    in_maps = []
    for c in range(N_CORES):
        xs = x[c * BL:(c + 1) * BL]                       # [128, t, F]
        xt = np.zeros((128, t_enc * 128), np.float32)
        xt[:F] = xs.transpose(2, 1, 0).reshape(F, -1)
        xt[F] = 1.0                                        # bias ones-row
        in_maps.append({"xT": _bf16(xt), **shared})
    return in_maps


def kernel(x, dec_in, eW0, eU0, eb0, eW1, eU1, eb1,
           dW0, dU0, db0, dW1, dU1, db1, denseW, denseb):
    x = np.asarray(x, np.float32)
    dec_in = np.asarray(dec_in, np.float32)
    # Generic-input guard: the on-device fast path folds eb0 and assumes the
    # remaining biases and dec_in are zero (true for this model's inputs).
    if (np.any(dec_in) or np.any(np.asarray(eb1)) or np.any(np.asarray(db0))
            or np.any(np.asarray(db1))):
        return _numpy_ref(x, dec_in, np.asarray(eW0), np.asarray(eU0),
                          np.asarray(eb0), np.asarray(eW1), np.asarray(eU1),
                          np.asarray(eb1), np.asarray(dW0), np.asarray(dU0),
                          np.asarray(db0), np.asarray(dW1), np.asarray(dU1),
                          np.asarray(db1), np.asarray(denseW),
                          np.asarray(denseb))

    t_enc, seg = x.shape[1], dec_in.shape[1]
    in_maps = make_in_maps(x, eW0, eU0, eb0, eW1, eU1, dU0, dW1, dU1,
                           denseW, t_enc)
    results = _run(in_maps, t_enc, seg)
    out = np.concatenate([results[c]["out"] for c in range(N_CORES)], axis=0)
    out = out + np.asarray(denseb, np.float32).reshape(1, 1)
    return out.reshape(B, seg, 1).astype(np.float32)
